# revision 4
# baseline (speedup 1.0000x reference)
"""Trainium2 Bass kernel for a 4-layer binarized MLP (BNN) with BatchNorm.

Reference computation (per layer l of 4):
    h = x @ sign(W_l).T ; BatchNorm(train-mode batch stats) ; sign() except last.

Strategy: tensor-parallel over output features across 8 NeuronCores, computed
in a transposed (feature-major) layout so BatchNorm reduces along the free
axis: H_l = sign(W_l)_c @ X_l with X_l = x_l.T. Each core owns 1/8 of each
layer's output features; BatchNorm is fully local (whole batch on-core); the
+-1 sign activations are all-gathered in fp8 between layers, strip-pipelined
(per-512-batch-strip collectives) so the next layer's GEMM overlaps the
gather. Layers 1-3 run DoubleRow fp8 matmuls (2 MACs/cell/cycle) - exact
integer arithmetic bit-matching the fp32 reference, including sign() ties
(h == mu resolves to exactly 0) via exact integer batch means. Layer 0
streams x as a 2048-scaled fp16 hi+lo split (two matmul passes; the pow-2
scale keeps the lo term out of fp16 subnormal flush and is undone exactly in
the PSUM evacuation)."""

import sys

sys.path.insert(0, "/opt/trn_rl_repo")

import numpy as np

import concourse.bass as bass  # noqa: F401
import concourse.tile as tile
from concourse import bacc, mybir
from concourse.bass_utils import run_bass_kernel_spmd

N_CORES = 8
B = 4096
KDIM = 4096
OUT_DIMS = [4096, 4096, 4096, 1024]
FS = [d // N_CORES for d in OUT_DIMS]
EPS = 1e-5
F32 = mybir.dt.float32
F16 = mybir.dt.float16
FP8 = mybir.dt.float8e4

NJ = B // 512
NK = KDIM // 128  # 32 fp16 k-tiles (layer 0)
NK2 = KDIM // 256  # 16 DoubleRow k-tiles (layers 1-3)
KG = 4  # k-tiles per DMA group, layer 0
KG2 = 2  # DoubleRow k-tiles per DMA group

_CACHED = {}


def _g_col(l, m):
    return [0, 8, 16, 24][l] + m


def _b_col(l, m):
    return [4, 12, 20, 25][l] + m


def build(repeat=1):
    nc = bacc.Bacc(
        "TRN2",
        target_bir_lowering=False,
        debug=False,
        enable_asserts=True,
        num_devices=N_CORES,
    )
    xt = nc.dram_tensor("xt", [KDIM, B], F32, kind="ExternalInput").ap()
    wts = [
        nc.dram_tensor(f"w{l}t", [KDIM, FS[l]], F32, kind="ExternalInput").ap()
        for l in range(4)
    ]
    gb = nc.dram_tensor("gb", [128, 26], F32, kind="ExternalInput").ap()
    out = nc.dram_tensor("out", [128, B], F32, kind="ExternalOutput").ap()

    with tile.TileContext(nc) as tc:
        with (
            tc.tile_pool(name="resident", bufs=1) as res,
            tc.tile_pool(name="wstage", bufs=2) as wstage,
            tc.tile_pool(name="xstage", bufs=2) as xstage,
            tc.tile_pool(name="xsplit", bufs=2) as xsplit,
            tc.tile_pool(name="sout", bufs=3) as soutp,
            tc.tile_pool(name="small", bufs=1) as small,
            tc.tile_pool(name="ps", bufs=8, space="PSUM") as pp,
            tc.tile_pool(name="dram", bufs=1, space="DRAM") as dram,
        ):
            gb_sb = small.tile([128, 26], F32, name="gb_sb", tag="gb_sb")
            nc.sync.dma_start(out=gb_sb, in_=gb)
            eps_sb = small.tile([128, 1], F32, name="eps_sb", tag="eps_sb")
            nc.vector.memset(eps_sb, EPS)

            # --- sign(W): layer 0 fp16 [128, FS] per 128-k-tile;
            #     layers 1-3 fp8 DoubleRow layout [128, 2, FS] per 256-k-tile ---
            sgn = {}
            for k in range(NK):
                wst = wstage.tile([128, FS[0]], F32, name=f"wst0_{k}", tag="wst")
                nc.sync.dma_start(out=wst, in_=wts[0][k * 128 : (k + 1) * 128, :])
                s_k = res.tile([128, FS[0]], F16, name=f"s0_{k}", tag=f"s0_{k}")
                nc.scalar.activation(
                    out=s_k, in_=wst, func=mybir.ActivationFunctionType.Sign
                )
                sgn[(0, k)] = s_k
            for l in range(1, 4):
                for k2 in range(NK2):
                    wst = wstage.tile(
                        [128, 2, FS[l]], F32, name=f"wst{l}_{k2}", tag="wst"
                    )
                    nc.sync.dma_start(
                        out=wst,
                        in_=wts[l][k2 * 256 : (k2 + 1) * 256, :].rearrange(
                            "(p two) m -> p two m", two=2
                        ),
                    )
                    s_k = res.tile(
                        [128, 2, FS[l]], FP8, name=f"s{l}_{k2}", tag=f"s{l}_{k2}"
                    )
                    nc.scalar.activation(
                        out=s_k, in_=wst, func=mybir.ActivationFunctionType.Sign
                    )
                    sgn[(l, k2)] = s_k

            H = [
                res.tile([128, B], F32, name=f"H_{m}", tag=f"H_{m}") for m in range(4)
            ]

            # --- per-(layer, strip) AllGather DRAM buffers ---
            ag_in = {
                (l, j): dram.tile(
                    [FS[l], 512], FP8, name=f"ag_in_{l}_{j}", tag=f"ag_in_{l}_{j}"
                )
                for l in range(3)
                for j in range(NJ)
            }
            ag_out = {
                (l, j): dram.tile(
                    [KDIM, 512], FP8, name=f"ag_out_{l}_{j}", tag=f"ag_out_{l}_{j}"
                )
                for l in range(3)
                for j in range(NJ)
            }

            for rep in range(repeat):
              for l in range(4):
                n_m = FS[l] // 128
                stats = [
                    small.tile([128, NJ, 6], F32, name=f"st_{l}_{m}", tag=f"st_{m}")
                    for m in range(n_m)
                ]
                csums = [
                    small.tile([128, NJ], F32, name=f"cs_{l}_{m}", tag=f"cs_{m}")
                    for m in range(n_m)
                ]
                for j in range(NJ):
                    psums = [
                        pp.tile([128, 512], F32, name=f"ps_{l}_{j}_{m}", tag="ps")
                        for m in range(n_m)
                    ]
                    if l == 0:
                        for kk in range(NK // KG):
                            xs = xstage.tile([128, KG, 512], F32, name="xs", tag="xs")
                            nc.sync.dma_start(
                                out=xs,
                                in_=xt[
                                    kk * KG * 128 : (kk + 1) * KG * 128,
                                    j * 512 : (j + 1) * 512,
                                ].rearrange("(four p) n -> p four n", p=128),
                            )
                            # scaled fp16 split: hi = fp16(2048*x),
                            # lo = fp16(2048*x - hi); keeps lo in fp16 normal
                            # range (unscaled x*2^-12 residuals would hit the
                            # fp16 subnormal flush). The 1/2048 is applied
                            # exactly (pow2) in the PSUM->SBUF copy.
                            xhi = xsplit.tile(
                                [128, KG, 512], F16, name="xhi", tag="xhi"
                            )
                            nc.vector.tensor_scalar_mul(xhi, xs, 2048.0)
                            xlo = xsplit.tile(
                                [128, KG, 512], F16, name="xlo", tag="xlo"
                            )
                            nc.vector.scalar_tensor_tensor(
                                out=xlo, in0=xs, scalar=2048.0, in1=xhi,
                                op0=mybir.AluOpType.mult,
                                op1=mybir.AluOpType.subtract,
                            )
                            for t in range(KG):
                                k = kk * KG + t
                                for m in range(n_m):
                                    lhsT = sgn[(0, k)][:, m * 128 : (m + 1) * 128]
                                    nc.tensor.matmul(
                                        psums[m], lhsT, xhi[:, t, :],
                                        start=(k == 0), stop=False,
                                    )
                                    nc.tensor.matmul(
                                        psums[m], lhsT, xlo[:, t, :],
                                        start=False, stop=(k == NK - 1),
                                    )
                    else:
                        for kk in range(NK2 // KG2):
                            xf = xstage.tile(
                                [128, KG2, 2, 512], FP8, name="xf", tag="xs"
                            )
                            nc.sync.dma_start(
                                out=xf,
                                in_=ag_out[(l - 1, j)][
                                    kk * KG2 * 256 : (kk + 1) * KG2 * 256, :
                                ].rearrange("(g p two) n -> p g two n", g=KG2, two=2),
                            )
                            for t in range(KG2):
                                k2 = kk * KG2 + t
                                for m in range(n_m):
                                    lhsT = sgn[(l, k2)][:, :, m * 128 : (m + 1) * 128]
                                    nc.tensor.matmul(
                                        psums[m], lhsT, xf[:, t, :, :],
                                        start=(k2 == 0), stop=(k2 == NK2 - 1),
                                        perf_mode=mybir.MatmulPerfMode.DoubleRow,
                                    )
                    for m in range(n_m):
                        hchunk = H[m][:, j * 512 : (j + 1) * 512]
                        nc.scalar.activation(
                            out=hchunk, in_=psums[m],
                            func=mybir.ActivationFunctionType.Copy,
                            scale=(1.0 / 2048.0) if l == 0 else 1.0,
                            accum_out=csums[m][:, j : j + 1],
                        )
                        nc.vector.bn_stats(out=stats[m][:, j, :], in_=hchunk)

                # --- BatchNorm epilogue ---
                a_ms, b_ms = [], []
                for m in range(n_m):
                    mv = small.tile([128, 2], F32, name=f"mv_{l}_{m}", tag=f"mv_{m}")
                    nc.vector.bn_aggr(out=mv, in_=stats[m])
                    # exact batch mean: bn_aggr's running mean is not exact on
                    # integer-valued h, which breaks sign() ties (h == mu) that
                    # the fp32 reference resolves to exactly 0. Chunk sums from
                    # accum_out are exact (integers < 2^24), as is *2^-12.
                    tot = small.tile([128, 1], F32, name=f"tot_{l}_{m}", tag=f"tot_{m}")
                    nc.vector.reduce_sum(out=tot, in_=csums[m], axis=mybir.AxisListType.X)
                    mu = small.tile([128, 1], F32, name=f"mu_{l}_{m}", tag=f"mu_{m}")
                    nc.vector.tensor_scalar_mul(mu, tot, 1.0 / 4096.0)
                    std = small.tile([128, 1], F32, name=f"sd_{l}_{m}", tag=f"sd_{m}")
                    nc.scalar.activation(
                        out=std, in_=mv[:, 1:2],
                        func=mybir.ActivationFunctionType.Sqrt,
                        bias=eps_sb, scale=1.0,
                    )
                    rstd = small.tile([128, 1], F32, name=f"r_{l}_{m}", tag=f"r_{m}")
                    nc.vector.reciprocal(out=rstd, in_=std)
                    a_m = small.tile([128, 1], F32, name=f"a_{l}_{m}", tag=f"a_{m}")
                    nc.vector.tensor_mul(
                        a_m, rstd, gb_sb[:, _g_col(l, m) : _g_col(l, m) + 1]
                    )
                    # t = h - mu (exact, matches reference's fp32 subtract)
                    nc.vector.tensor_scalar_sub(H[m], H[m], mu)
                    a_ms.append(a_m)
                    b_ms.append(gb_sb[:, _b_col(l, m) : _b_col(l, m) + 1])

                if l < 3:
                    for j in range(NJ):
                        for m in range(n_m):
                            s_mj = soutp.tile([128, 512], FP8, name="s_mj", tag="s_mj")
                            nc.scalar.activation(
                                out=s_mj, in_=H[m][:, j * 512 : (j + 1) * 512],
                                func=mybir.ActivationFunctionType.Sign,
                                bias=b_ms[m], scale=a_ms[m],
                            )
                            nc.sync.dma_start(
                                out=ag_in[(l, j)][m * 128 : (m + 1) * 128, :],
                                in_=s_mj,
                            )
                        nc.gpsimd.collective_compute(
                            "AllGather",
                            mybir.AluOpType.bypass,
                            replica_groups=[list(range(N_CORES))],
                            ins=[ag_in[(l, j)][:]],
                            outs=[ag_out[(l, j)][:]],
                        )
                else:
                    o_sb = soutp.tile([128, B], F32, name="o_sb", tag="o_sb", bufs=1)
                    nc.scalar.activation(
                        out=o_sb, in_=H[0],
                        func=mybir.ActivationFunctionType.Identity,
                        bias=b_ms[0], scale=a_ms[0],
                    )
                    nc.sync.dma_start(out=out, in_=o_sb)

    nc.compile()
    return nc


def _prep_inputs(x, Ws, gs, bs):
    xt = np.ascontiguousarray(x.T)
    in_maps = []
    for c in range(N_CORES):
        m = {"xt": xt}
        for l in range(4):
            f0 = c * FS[l]
            m[f"w{l}t"] = np.ascontiguousarray(Ws[l].T[:, f0 : f0 + FS[l]])
        gbp = np.zeros((128, 26), np.float32)
        for l in range(4):
            n_m = FS[l] // 128
            gsh = gs[l][c * FS[l] : (c + 1) * FS[l]].reshape(n_m, 128).T
            bsh = bs[l][c * FS[l] : (c + 1) * FS[l]].reshape(n_m, 128).T
            gbp[:, _g_col(l, 0) : _g_col(l, 0) + n_m] = gsh
            gbp[:, _b_col(l, 0) : _b_col(l, 0) + n_m] = bsh
        m["gb"] = gbp
        in_maps.append(m)
    return in_maps


def kernel(x, W0, W1, W2, W3, g0, b0, g1, b1, g2, b2, g3, b3):
    x = np.asarray(x, np.float32)
    Ws = [np.asarray(w, np.float32) for w in (W0, W1, W2, W3)]
    gs = [np.asarray(g, np.float32) for g in (g0, g1, g2, g3)]
    bs = [np.asarray(b, np.float32) for b in (b0, b1, b2, b3)]

    if "nc" not in _CACHED:
        _CACHED["nc"] = build()
    nc = _CACHED["nc"]

    in_maps = _prep_inputs(x, Ws, gs, bs)
    res = run_bass_kernel_spmd(nc, in_maps, core_ids=list(range(N_CORES)))
    full = np.concatenate([res.results[c]["out"] for c in range(N_CORES)], axis=0)
    return np.ascontiguousarray(full.T)
